# revision 20
# baseline (speedup 1.0000x reference)
"""Trainium2 Bass kernel for nn_DiscreteQKTRBlock (sparse 3x3x3 neighborhood
attention with a discrete codebook).

Strategy (data-parallel over points, 8 cores):

The reference's discrete-codebook STE path collapses algebraically:
    dq[i]   = codebook * choice[i]           (forward value of the STE)
    s[k,i]  = dq[i] . dq[nbr[k,i]] = ||codebook||^2 * choice[i] * choice[nbr[k,i]]
so the [N,128] per-offset dot products reduce to scalar products of a single
[N] vector `choice`.  Phases:

  A) each core computes q (sparse 27-offset conv via indirect row gathers of
     x), then choice' = sqrt(cb2)*choice for its 12544-point shard
  B) AllGather choice' (50KB/core); every core builds the full value table
     Tv[i] = [v_f(i) (128) | choice'(i)]  (v_f = relu(x@Wv*g+b)+pos)
  C) gather Tv rows for the 27 neighbors of each own point, masked softmax
     over offsets, weighted accumulation, output projection + residual.

Masking is folded in host-side: masked/padded neighbors get index Z=100000
which points at an all-zero table row, plus a -1e9 additive score bias.

All weight-affine folds (gamma/beta, codebook scaling into W_choice, bias
rows appended to coords) are host-side weight-space transforms only.
"""
import sys
sys.path.insert(0, "/opt/trn_rl_repo")
import numpy as np
import ml_dtypes

from concourse import bass, bacc, mybir
import concourse.tile as tile
from concourse.bass_utils import run_bass_kernel_spmd
from concourse.masks import make_identity

F32 = mybir.dt.float32
BF16 = mybir.dt.bfloat16
FP16 = mybir.dt.float16
I32 = mybir.dt.int32

N = 100000
P = 128
VEC = 16
K = 27
NEG = -1e9
NCORE = 8
NSH = 12544                # points per core (98 tiles of 128)
TO = NSH // P              # 98 own tiles
PAD_N = 100096             # 782 * 128  (full table rows incl. zero row)
TA = PAD_N // P            # 782 build tiles
Z = N                      # zero-row index for masked/padded neighbors
D = 129                    # Tv row: 128 v_f + 1 choice

_CACHE = {}


def _build_nc(kts):
    nc = bacc.Bacc(num_devices=NCORE, dynamic_dma_scratch_size=131072)

    # ---------------- inputs ----------------
    xT16 = nc.declare_dram_parameter("xT16", [P, PAD_N], FP16, isOutput=False)
    coordsT4 = nc.declare_dram_parameter("coordsT4", [4, PAD_N], F32, isOutput=False)
    xT_own = nc.declare_dram_parameter("xT_own", [P, NSH], F32, isOutput=False)
    idx_own = nc.declare_dram_parameter("idx_own", [NSH, K], I32, isOutput=False)
    idxa_own = nc.declare_dram_parameter("idxa_own", [NSH, K], I32, isOutput=False)
    bias_own = nc.declare_dram_parameter("bias_own", [NSH, K], F32, isOutput=False)
    w_q = nc.declare_dram_parameter("w_q", [P, K * VEC], FP16, isOutput=False)
    qg_in = nc.declare_dram_parameter("qg", [VEC, 1], F32, isOutput=False)
    qb_in = nc.declare_dram_parameter("qb", [VEC, 1], F32, isOutput=False)
    wcc_in = nc.declare_dram_parameter("wcc", [VEC, P], F32, isOutput=False)
    bch_in = nc.declare_dram_parameter("bch", [1, P], F32, isOutput=False)
    wv_in = nc.declare_dram_parameter("wv", [P, P], FP16, isOutput=False)
    vbeta_in = nc.declare_dram_parameter("vbeta", [1, P], F32, isOutput=False)
    wpos4_in = nc.declare_dram_parameter("wpos4", [4, VEC], F32, isOutput=False)
    wo_in = nc.declare_dram_parameter("wo", [P, P], F32, isOutput=False)
    obeta_in = nc.declare_dram_parameter("obeta", [P, 1], F32, isOutput=False)

    outT = nc.declare_dram_parameter("outT", [P, NSH], F32, isOutput=True)

    with tile.TileContext(nc) as tc:
        with tc.tile_pool(name="const", bufs=1) as cpool, \
             tc.tile_pool(name="work", bufs=1) as wpool, \
             tc.tile_pool(name="dram", bufs=1, space="DRAM") as dpool:

            # resident constants
            wq_sb = cpool.tile([P, K * VEC], FP16)
            nc.sync.dma_start(out=wq_sb[:], in_=w_q[:, :])
            qg_sb = cpool.tile([VEC, 1], F32)
            nc.sync.dma_start(out=qg_sb[:], in_=qg_in[:, :])
            qb_sb = cpool.tile([VEC, 1], F32)
            nc.sync.dma_start(out=qb_sb[:], in_=qb_in[:, :])
            wcc_sb = cpool.tile([VEC, P], F32)
            nc.sync.dma_start(out=wcc_sb[:], in_=wcc_in[:, :])
            bch_sb = cpool.tile([1, P], F32)
            nc.sync.dma_start(out=bch_sb[:], in_=bch_in[:, :])
            wv_sb = cpool.tile([P, P], FP16)
            nc.sync.dma_start(out=wv_sb[:], in_=wv_in[:, :])
            vbeta_sb = cpool.tile([1, P], F32)
            nc.sync.dma_start(out=vbeta_sb[:], in_=vbeta_in[:, :])
            wpos4_sb = cpool.tile([4, VEC], F32)
            nc.sync.dma_start(out=wpos4_sb[:], in_=wpos4_in[:, :])
            wo_sb = cpool.tile([P, P], F32)
            nc.sync.dma_start(out=wo_sb[:], in_=wo_in[:, :])
            obeta_sb = cpool.tile([P, 1], F32)
            nc.sync.dma_start(out=obeta_sb[:], in_=obeta_in[:, :])

            ident32 = cpool.tile([P, P], F32)
            make_identity(nc, ident32[:])


            ones_row = cpool.tile([1, P], F32)
            nc.vector.memset(ones_row[:], 1.0)

            strip = cpool.tile([P, TO], F32)        # own choice' per tile col
            choice_all = cpool.tile([P, TA + 2], F32)  # wrapped full choice'

            Tv = dpool.tile([PAD_N, D], F32)
            Yf = dpool.tile([PAD_N * K, VEC], FP16)
            cc_in = dpool.tile([P, TO], F32)
            cc_out = dpool.tile([NCORE, P, TO], F32, addr_space="Shared")

            # ---------------- all per-phase pools (opened up-front so phases overlap) ----
            from contextlib import ExitStack
            _stk = ExitStack()
            ipool = _stk.enter_context(tc.tile_pool(name="a_idx", bufs=2))
            gpool = _stk.enter_context(tc.tile_pool(name="a_xg", bufs=32))
            tpool = _stk.enter_context(tc.tile_pool(name="a_xgT", bufs=6))
            pspool = _stk.enter_context(tc.tile_pool(name="a_ps", bufs=1, space="PSUM"))
            ps2pool = _stk.enter_context(tc.tile_pool(name="a_ps2", bufs=1, space="PSUM"))
            ypool = _stk.enter_context(tc.tile_pool(name="y_x", bufs=4))
            ysb = _stk.enter_context(tc.tile_pool(name="y_sb", bufs=3))
            yps = _stk.enter_context(tc.tile_pool(name="y_ps", bufs=2, space="PSUM"))
            bxpool = _stk.enter_context(tc.tile_pool(name="b_x", bufs=4))
            btvpool = _stk.enter_context(tc.tile_pool(name="b_tv", bufs=4))
            bpspool = _stk.enter_context(tc.tile_pool(name="b_ps", bufs=1, space="PSUM"))
            cipool = _stk.enter_context(tc.tile_pool(name="c_idx", bufs=2))
            cgpool = _stk.enter_context(tc.tile_pool(name="c_g", bufs=3))
            cspool = _stk.enter_context(tc.tile_pool(name="c_s", bufs=2))
            cpspool = _stk.enter_context(tc.tile_pool(name="c_ps", bufs=1, space="PSUM"))

            # ---------------- phase Y: Y = x @ Wq_all ----------------
            with nc.named_scope("phaseY"):
                for g in range(TA):
                    yx_t = ypool.tile([P, P], FP16, tag="yx")
                    nc.sync.dma_start(out=yx_t[:],
                                      in_=xT16[:, g * P:(g + 1) * P])
                    y_ps = yps.tile([P, K * VEC], F32, tag="yps")
                    nc.tensor.matmul(out=y_ps[:], lhsT=yx_t[:], rhs=wq_sb[:],
                                     start=True, stop=True)
                    y_sb_t = ysb.tile([P, K * VEC], FP16, tag="ysb")
                    nc.vector.tensor_copy(out=y_sb_t[:], in_=y_ps[:])
                    nc.scalar.dma_start(
                        out=Yf[g * P * K:(g + 1) * P * K, :].rearrange(
                            "(p k) v -> p (k v)", p=P),
                        in_=y_sb_t[:])

            # ---------------- phase A: q + choice on own shard ----------------
            with nc.named_scope("phaseA"):
                if True:
                    for t in range(TO):
                        KT = kts[t]
                        idxa_t = ipool.tile([P, KT], I32)
                        nc.sync.dma_start(out=idxa_t[:],
                                          in_=idxa_own[t * P:(t + 1) * P, 0:KT])
                        qacc = tpool.tile([P, VEC], F32, tag="qacc")
                        for k in range(KT):
                            yg = gpool.tile([P, VEC], FP16, tag="yg")
                            nc.gpsimd.indirect_dma_start(
                                out=yg[:], out_offset=None, in_=Yf[:, :],
                                in_offset=bass.IndirectOffsetOnAxis(
                                    ap=idxa_t[:, k:k + 1], axis=0))
                            if k == 0:
                                nc.vector.tensor_copy(out=qacc[:], in_=yg[:])
                            else:
                                nc.vector.tensor_tensor(
                                    out=qacc[:], in0=qacc[:], in1=yg[:],
                                    op=mybir.AluOpType.add)
                        q_ps = pspool.tile([VEC, P], F32, tag="qT")
                        nc.tensor.matmul(out=q_ps[:], lhsT=qacc[:],
                                         rhs=ident32[:], start=True, stop=True)
                        qf = tpool.tile([VEC, P], F32, tag="qf")
                        nc.scalar.activation(
                            out=qf[:], in_=q_ps[:],
                            func=mybir.ActivationFunctionType.Relu,
                            bias=qb_sb[:, 0:1], scale=qg_sb[:, 0:1])
                        t_ps = ps2pool.tile([P, P], F32, tag="tch")
                        nc.tensor.matmul(out=t_ps[:], lhsT=qf[:], rhs=wcc_sb[:],
                                         start=True, stop=False)
                        nc.tensor.matmul(out=t_ps[:], lhsT=ones_row[:],
                                         rhs=bch_sb[:], start=False, stop=True)
                        scratch = tpool.tile([P, P], F32, tag="scr")
                        nc.scalar.activation(
                            out=scratch[:], in_=t_ps[:],
                            func=mybir.ActivationFunctionType.Relu,
                            accum_out=strip[:, t:t + 1])

            # ---------------- phase B: build Tv table ----------------
            with nc.named_scope("phaseB"):
                if True:
                    for g in range(TA):
                        xt_t = bxpool.tile([P, P], FP16, tag="xt")
                        nc.sync.dma_start(out=xt_t[:],
                                          in_=xT16[:, g * P:(g + 1) * P])
                        c4_t = bxpool.tile([4, P], F32, tag="c4")
                        nc.sync.dma_start(out=c4_t[:],
                                          in_=coordsT4[:, g * P:(g + 1) * P])
                        v_ps = bpspool.tile([P, P], F32, tag="vps")
                        nc.tensor.matmul(out=v_ps[:], lhsT=xt_t[:], rhs=wv_sb[:],
                                         start=True, stop=False)
                        nc.tensor.matmul(out=v_ps[:], lhsT=ones_row[:],
                                         rhs=vbeta_sb[:], start=False, stop=True)
                        p_ps = bpspool.tile([P, VEC], F32, tag="pps")
                        nc.tensor.matmul(out=p_ps[:], lhsT=c4_t[:],
                                         rhs=wpos4_sb[:], start=True, stop=True)
                        tv_t = btvpool.tile([P, D], F32, tag="tv")
                        nc.scalar.activation(
                            out=tv_t[:, 0:P], in_=v_ps[:],
                            func=mybir.ActivationFunctionType.Relu)
                        pos_bc = bass.AP(p_ps.tensor, p_ps[:].offset,
                                         [p_ps[:].ap[0], (1, VEC), (0, P // VEC)])
                        nc.vector.tensor_tensor(
                            out=tv_t[:, 0:P], in0=tv_t[:, 0:P], in1=pos_bc,
                            op=mybir.AluOpType.add)
                        nc.vector.memset(tv_t[:, P:D], 0.0)
                        nc.scalar.dma_start(out=Tv[g * P:(g + 1) * P, :],
                                            in_=tv_t[:])

            # ---------------- allgather choice ----------------
            with nc.named_scope("gather_choice"):
                nc.sync.dma_start(out=cc_in[:], in_=strip[:])
                nc.gpsimd.collective_compute(
                    "AllGather", mybir.AluOpType.bypass,
                    replica_groups=[list(range(NCORE))],
                    ins=[cc_in.opt()], outs=[cc_out.opt()])
                nc.sync.dma_start(
                    out=choice_all[:, 0:NCORE * TO].rearrange(
                        "p (r t) -> p r t", r=NCORE),
                    in_=cc_out[:, :, :].rearrange("r p t -> p r t"))

            # ---------------- phase B2: patch choice column into Tv ----------------
            with nc.named_scope("phaseB2"):
                GRP = 16
                for g0 in range(0, TA, GRP):
                    gn = min(GRP, TA - g0)
                    dst = bass.AP(Tv.tensor, g0 * P * D + P,
                                  [(D, P), (D * P, gn)])
                    nc.sync.dma_start(out=dst,
                                      in_=choice_all[:, g0:g0 + gn])

            # ---------------- phase C: scores, softmax, aggregate, out ----------------
            with nc.named_scope("phaseC"):
                if True:
                    for t in range(TO):
                        KT = kts[t]
                        idx_t = cipool.tile([P, KT], I32, tag="idx")
                        nc.sync.dma_start(out=idx_t[:],
                                          in_=idx_own[t * P:(t + 1) * P, 0:KT])
                        bias_t = cipool.tile([P, KT], F32, tag="bias")
                        nc.sync.dma_start(out=bias_t[:],
                                          in_=bias_own[t * P:(t + 1) * P, 0:KT])
                        g_all = cgpool.tile([P, KT * D], F32, tag="gall")
                        for k in range(KT):
                            nc.gpsimd.indirect_dma_start(
                                out=g_all[:, k * D:(k + 1) * D],
                                out_offset=None, in_=Tv[:, :],
                                in_offset=bass.IndirectOffsetOnAxis(
                                    ap=idx_t[:, k:k + 1], axis=0))
                        chg = g_all[:].rearrange("p (k d) -> p k d", k=KT)[:, :, P]
                        s_t = cspool.tile([P, KT], F32, tag="s")
                        nc.vector.scalar_tensor_tensor(
                            out=s_t[:], in0=chg, scalar=strip[:, t:t + 1],
                            in1=bias_t[:], op0=mybir.AluOpType.mult,
                            op1=mybir.AluOpType.add)
                        negmax = cspool.tile([P, 1], F32, tag="nm")
                        nc.vector.tensor_reduce(
                            out=negmax[:], in_=s_t[:], axis=mybir.AxisListType.X,
                            op=mybir.AluOpType.max, negate=True)
                        e_t = cspool.tile([P, KT], F32, tag="e")
                        esum = cspool.tile([P, 1], F32, tag="es")
                        nc.scalar.activation(
                            out=e_t[:], in_=s_t[:],
                            func=mybir.ActivationFunctionType.Exp,
                            bias=negmax[:, 0:1], scale=1.0,
                            accum_out=esum[:, 0:1])
                        rs = cspool.tile([P, 1], F32, tag="rs")
                        nc.vector.reciprocal(out=rs[:], in_=esum[:])
                        w_t = cspool.tile([P, KT], F32, tag="w")
                        nc.vector.tensor_scalar_mul(out=w_t[:], in0=e_t[:],
                                                    scalar1=rs[:, 0:1])
                        acc = cspool.tile([P, P], F32, tag="acc")
                        for k in range(KT):
                            vsl = g_all[:, k * D:k * D + P]
                            if k == 0:
                                nc.vector.tensor_scalar_mul(
                                    out=acc[:], in0=vsl, scalar1=w_t[:, 0:1])
                            else:
                                nc.vector.scalar_tensor_tensor(
                                    out=acc[:], in0=vsl, scalar=w_t[:, k:k + 1],
                                    in1=acc[:], op0=mybir.AluOpType.mult,
                                    op1=mybir.AluOpType.add)
                        tr2 = cpspool.tile([P, P], F32, tag="tr2")
                        nc.tensor.transpose(out=tr2[:], in_=acc[:],
                                            identity=ident32[:])
                        aggT = cspool.tile([P, P], F32, tag="aggT")
                        nc.vector.tensor_copy(out=aggT[:], in_=tr2[:])
                        o_ps = cpspool.tile([P, P], F32, tag="ops")
                        nc.tensor.matmul(out=o_ps[:], lhsT=wo_sb[:], rhs=aggT[:],
                                         start=True, stop=True)
                        oT = cspool.tile([P, P], F32, tag="oT")
                        nc.scalar.activation(
                            out=oT[:], in_=o_ps[:],
                            func=mybir.ActivationFunctionType.Relu,
                            bias=obeta_sb[:, 0:1], scale=1.0)
                        xo_t = cspool.tile([P, P], F32, tag="xo")
                        nc.sync.dma_start(out=xo_t[:],
                                          in_=xT_own[:, t * P:(t + 1) * P])
                        res_t = cspool.tile([P, P], F32, tag="res")
                        nc.vector.tensor_tensor(out=res_t[:], in0=oT[:],
                                                in1=xo_t[:],
                                                op=mybir.AluOpType.add)
                        nc.scalar.dma_start(out=outT[:, t * P:(t + 1) * P],
                                            in_=res_t[:])
            _stk.close()

    nc.finalize()
    return nc


def _prep(inputs):
    x = np.asarray(inputs["x"], np.float32)
    coords = np.asarray(inputs["coords"], np.float32)
    W_q = np.asarray(inputs["W_q"], np.float32)
    q_gamma = np.asarray(inputs["q_gamma"], np.float32)
    q_beta = np.asarray(inputs["q_beta"], np.float32)
    W_v = np.asarray(inputs["W_v"], np.float32)
    v_gamma = np.asarray(inputs["v_gamma"], np.float32)
    v_beta = np.asarray(inputs["v_beta"], np.float32)
    codebook = np.asarray(inputs["codebook"], np.float32)
    W_choice = np.asarray(inputs["W_choice"], np.float32)
    b_choice = np.asarray(inputs["b_choice"], np.float32)
    W_pos = np.asarray(inputs["W_pos"], np.float32)
    b_pos = np.asarray(inputs["b_pos"], np.float32)
    W_out = np.asarray(inputs["W_out"], np.float32)
    out_gamma = np.asarray(inputs["out_gamma"], np.float32)
    out_beta = np.asarray(inputs["out_beta"], np.float32)
    nbr_idx = np.asarray(inputs["nbr_idx"], np.int32)
    nbr_mask = np.asarray(inputs["nbr_mask"], np.int32)

    n = x.shape[0]
    assert n == N

    NTOT = NCORE * NSH                    # 100352 padded rows
    # ---- valid-degree sort (per core shard) → global relabeling ----
    mask_pad = np.zeros((K, NTOT), bool)
    mask_pad[:, :n] = nbr_mask > 0
    deg = mask_pad.sum(0)
    orders = []
    degs_sorted = np.empty((NCORE, NSH), np.int64)
    for r in range(NCORE):
        sl = slice(r * NSH, (r + 1) * NSH)
        o = np.argsort(-deg[sl], kind="stable")
        orders.append(o)
        degs_sorted[r] = deg[sl][o]
    kts = tuple(int(max(1, degs_sorted[:, t * P:(t + 1) * P].max()))
                for t in range(TO))
    perm_full = np.concatenate([r * NSH + orders[r] for r in range(NCORE)])
    inv = np.empty(NTOT, np.int64)
    inv[perm_full] = np.arange(NTOT)

    # ---- permuted global tables ----
    xp = np.zeros((NTOT, P), np.float32)
    xp[:n] = x
    xp2 = xp[perm_full]
    cp = np.zeros((NTOT, 3), np.float32)
    cp[:n] = coords
    cp2 = cp[perm_full]

    xT16 = np.ascontiguousarray(xp2[:PAD_N].T.astype(np.float16))
    coordsT4 = np.ones((4, PAD_N), np.float32)
    coordsT4[:3] = cp2[:PAD_N].T

    # ---- weight folds ----
    cb2 = float(np.dot(codebook, codebook))
    scb = np.sqrt(cb2).astype(np.float32)
    wcp = codebook[:, None] * W_choice
    wcc = scb * wcp.reshape(VEC, P // VEC, P).sum(1)
    bch = (scb * b_choice)[None, :]
    wv = (W_v * v_gamma[None, :]).astype(np.float16)
    wpos4 = np.concatenate([W_pos, b_pos[None, :]], axis=0)
    wq_flat = np.ascontiguousarray(
        W_q.transpose(1, 0, 2).reshape(P, K * VEC)).astype(np.float16)
    wo = W_out * out_gamma[None, :]

    # ---- per-slot idx/bias in NEW row ids, compacted valid-first ----
    idx_new = np.full((K, NTOT), Z, np.int32)
    idx_new[:, :n] = np.where(nbr_mask > 0, inv[nbr_idx], Z).astype(np.int32)
    bias_pad = np.full((K, NTOT), np.float32(NEG), np.float32)
    bias_pad[:, :n] = np.where(nbr_mask > 0, 0.0, NEG).astype(np.float32)
    korder = np.argsort(~mask_pad, axis=0, kind="stable")   # valid ks first
    idx_new = np.take_along_axis(idx_new, korder, axis=0)
    bias_pad = np.take_along_axis(bias_pad, korder, axis=0)
    # phase-A flat Y indices: neighbor_row*27 + original k (Z*27 for padding)
    idxa = np.where(idx_new != Z, idx_new.astype(np.int64) * K + korder,
                    Z * K).astype(np.int32)
    # permute slot-grid columns to sorted point order
    idx_new = idx_new[:, perm_full]
    bias_pad = bias_pad[:, perm_full]
    idxa = idxa[:, perm_full]

    shared = dict(xT16=xT16, coordsT4=coordsT4,
                  w_q=wq_flat,
                  qg=q_gamma[:, None], qb=q_beta[:, None],
                  wcc=wcc, bch=bch, wv=wv,
                  vbeta=v_beta[None, :],
                  wpos4=wpos4, wo=wo, obeta=out_beta[:, None])
    in_maps = []
    for r in range(NCORE):
        sl = slice(r * NSH, (r + 1) * NSH)
        m = dict(shared)
        m["xT_own"] = np.ascontiguousarray(xp2[sl].T)
        m["idx_own"] = np.ascontiguousarray(idx_new[:, sl].T)
        m["idxa_own"] = np.ascontiguousarray(idxa[:, sl].T)
        m["bias_own"] = np.ascontiguousarray(bias_pad[:, sl].T)
        in_maps.append(m)
    return in_maps, kts, orders


def prepare(inputs):
    in_maps, kts, orders = _prep(inputs)
    if _CACHE.get("kts") != kts:
        _CACHE["nc"] = _build_nc(kts)
        _CACHE["kts"] = kts
    return _CACHE["nc"], in_maps, orders


def assemble(results, orders):
    out = np.empty((NCORE * NSH, P), np.float32)
    for r in range(NCORE):
        out[r * NSH + orders[r]] = results[r]["outT"].T
    return np.ascontiguousarray(out[:N])


def kernel(**inputs):
    nc, in_maps, orders = prepare(inputs)
    res = run_bass_kernel_spmd(nc, in_maps, list(range(NCORE)))
    return assemble(res.results, orders)


if __name__ == "__main__":
    rng = np.random.default_rng(0)
    ins = dict(
        x=rng.standard_normal((N, P)).astype(np.float32),
        coords=(rng.random((N, 3)) * 100).astype(np.float32),
        W_q=rng.standard_normal((K, P, VEC)).astype(np.float32) * (P * K) ** -0.5,
        q_gamma=np.ones(VEC, np.float32), q_beta=np.zeros(VEC, np.float32),
        W_v=rng.standard_normal((P, P)).astype(np.float32) * P ** -0.5,
        v_gamma=np.ones(P, np.float32), v_beta=np.zeros(P, np.float32),
        codebook=rng.standard_normal(P).astype(np.float32) * 0.1,
        W_choice=rng.standard_normal((P, P)).astype(np.float32) * P ** -0.5,
        b_choice=np.zeros(P, np.float32),
        W_pos=rng.standard_normal((3, VEC)).astype(np.float32) * 3 ** -0.5,
        b_pos=np.zeros(VEC, np.float32),
        W_out=rng.standard_normal((P, P)).astype(np.float32) * P ** -0.5,
        out_gamma=np.ones(P, np.float32), out_beta=np.zeros(P, np.float32),
        nbr_idx=rng.integers(0, N, (K, N)).astype(np.int32),
        nbr_mask=rng.integers(0, 2, (K, N)).astype(np.int32),
    )
    out = kernel(**ins)
    print("kernel output", out.shape, out.dtype)


# revision 21
# speedup vs baseline: 1.1637x; 1.1637x over previous
"""Trainium2 Bass kernel for nn_DiscreteQKTRBlock (sparse 3x3x3 neighborhood
attention with a discrete codebook).

Strategy (data-parallel over points, 8 cores):

The reference's discrete-codebook STE path collapses algebraically:
    dq[i]   = codebook * choice[i]           (forward value of the STE)
    s[k,i]  = dq[i] . dq[nbr[k,i]] = ||codebook||^2 * choice[i] * choice[nbr[k,i]]
so the [N,128] per-offset dot products reduce to scalar products of a single
[N] vector `choice`.  Phases:

  A) each core computes q (sparse 27-offset conv via indirect row gathers of
     x), then choice' = sqrt(cb2)*choice for its 12544-point shard
  B) AllGather choice' (50KB/core); every core builds the full value table
     Tv[i] = [v_f(i) (128) | choice'(i)]  (v_f = relu(x@Wv*g+b)+pos)
  C) gather Tv rows for the 27 neighbors of each own point, masked softmax
     over offsets, weighted accumulation, output projection + residual.

Masking is folded in host-side: masked/padded neighbors get index Z=100000
which points at an all-zero table row, plus a -1e9 additive score bias.

All weight-affine folds (gamma/beta, codebook scaling into W_choice, bias
rows appended to coords) are host-side weight-space transforms only.
"""
import sys
sys.path.insert(0, "/opt/trn_rl_repo")
import numpy as np
import ml_dtypes

from concourse import bass, bacc, mybir
import concourse.tile as tile
from concourse.bass_utils import run_bass_kernel_spmd
from concourse.masks import make_identity

F32 = mybir.dt.float32
BF16 = mybir.dt.bfloat16
FP16 = mybir.dt.float16
I32 = mybir.dt.int32

N = 100000
P = 128
VEC = 16
K = 27
NEG = -1e9
NCORE = 8
NSH = 12544                # points per core (98 tiles of 128)
TO = NSH // P              # 98 own tiles
PAD_N = 100096             # 782 * 128  (full table rows incl. zero row)
TA = PAD_N // P            # 782 build tiles
Z = N                      # zero-row index for masked/padded neighbors
D = 129                    # Tv row: 128 v_f + 1 choice

_CACHE = {}


def _build_nc(kts):
    nc = bacc.Bacc(num_devices=NCORE, dynamic_dma_scratch_size=131072)

    # ---------------- inputs ----------------
    xT16 = nc.declare_dram_parameter("xT16", [P, PAD_N], FP16, isOutput=False)
    coordsT4 = nc.declare_dram_parameter("coordsT4", [4, PAD_N], F32, isOutput=False)
    xT_own = nc.declare_dram_parameter("xT_own", [P, NSH], F32, isOutput=False)
    idx_own = nc.declare_dram_parameter("idx_own", [NSH, K], I32, isOutput=False)
    idxa_own = nc.declare_dram_parameter("idxa_own", [NSH, K], I32, isOutput=False)
    bias_own = nc.declare_dram_parameter("bias_own", [NSH, K], F32, isOutput=False)
    w_q = nc.declare_dram_parameter("w_q", [P, K * VEC], FP16, isOutput=False)
    qg_in = nc.declare_dram_parameter("qg", [VEC, 1], F32, isOutput=False)
    qb_in = nc.declare_dram_parameter("qb", [VEC, 1], F32, isOutput=False)
    wcc_in = nc.declare_dram_parameter("wcc", [VEC, P], F32, isOutput=False)
    bch_in = nc.declare_dram_parameter("bch", [1, P], F32, isOutput=False)
    wv_in = nc.declare_dram_parameter("wv", [P, P], FP16, isOutput=False)
    vbeta_in = nc.declare_dram_parameter("vbeta", [1, P], F32, isOutput=False)
    wpos4_in = nc.declare_dram_parameter("wpos4", [4, VEC], F32, isOutput=False)
    wo_in = nc.declare_dram_parameter("wo", [P, P], F32, isOutput=False)
    obeta_in = nc.declare_dram_parameter("obeta", [P, 1], F32, isOutput=False)

    outT = nc.declare_dram_parameter("outT", [P, NSH], F32, isOutput=True)

    with tile.TileContext(nc) as tc:
        with tc.tile_pool(name="const", bufs=1) as cpool, \
             tc.tile_pool(name="work", bufs=1) as wpool, \
             tc.tile_pool(name="dram", bufs=1, space="DRAM") as dpool:

            # resident constants
            wq_sb = cpool.tile([P, K * VEC], FP16)
            nc.sync.dma_start(out=wq_sb[:], in_=w_q[:, :])
            qg_sb = cpool.tile([VEC, 1], F32)
            nc.sync.dma_start(out=qg_sb[:], in_=qg_in[:, :])
            qb_sb = cpool.tile([VEC, 1], F32)
            nc.sync.dma_start(out=qb_sb[:], in_=qb_in[:, :])
            wcc_sb = cpool.tile([VEC, P], F32)
            nc.sync.dma_start(out=wcc_sb[:], in_=wcc_in[:, :])
            bch_sb = cpool.tile([1, P], F32)
            nc.sync.dma_start(out=bch_sb[:], in_=bch_in[:, :])
            wv_sb = cpool.tile([P, P], FP16)
            nc.sync.dma_start(out=wv_sb[:], in_=wv_in[:, :])
            vbeta_sb = cpool.tile([1, P], F32)
            nc.sync.dma_start(out=vbeta_sb[:], in_=vbeta_in[:, :])
            wpos4_sb = cpool.tile([4, VEC], F32)
            nc.sync.dma_start(out=wpos4_sb[:], in_=wpos4_in[:, :])
            wo_sb = cpool.tile([P, P], F32)
            nc.sync.dma_start(out=wo_sb[:], in_=wo_in[:, :])
            obeta_sb = cpool.tile([P, 1], F32)
            nc.sync.dma_start(out=obeta_sb[:], in_=obeta_in[:, :])

            ident32 = cpool.tile([P, P], F32)
            make_identity(nc, ident32[:])


            ones_row = cpool.tile([1, P], F32)
            nc.vector.memset(ones_row[:], 1.0)

            strip = cpool.tile([P, TO], F32)        # own choice' per tile col
            choice_all = cpool.tile([P, TA + 2], F32)  # wrapped full choice'

            Tv = dpool.tile([PAD_N, D], F32)
            Yf = dpool.tile([PAD_N * K, VEC], FP16)
            cc_in = dpool.tile([P, TO], F32)
            cc_out = dpool.tile([NCORE, P, TO], F32, addr_space="Shared")

            # ---------------- all per-phase pools (opened up-front so phases overlap) ----
            from contextlib import ExitStack
            _stk = ExitStack()
            ipool = _stk.enter_context(tc.tile_pool(name="a_idx", bufs=2))
            gpool = _stk.enter_context(tc.tile_pool(name="a_xg", bufs=32))
            tpool = _stk.enter_context(tc.tile_pool(name="a_xgT", bufs=6))
            pspool = _stk.enter_context(tc.tile_pool(name="a_ps", bufs=1, space="PSUM"))
            ps2pool = _stk.enter_context(tc.tile_pool(name="a_ps2", bufs=1, space="PSUM"))
            ypool = _stk.enter_context(tc.tile_pool(name="y_x", bufs=6))
            ysb = _stk.enter_context(tc.tile_pool(name="y_sb", bufs=10))
            yps = _stk.enter_context(tc.tile_pool(name="y_ps", bufs=2, space="PSUM"))
            bxpool = _stk.enter_context(tc.tile_pool(name="b_x", bufs=4))
            btvpool = _stk.enter_context(tc.tile_pool(name="b_tv", bufs=10))
            bpspool = _stk.enter_context(tc.tile_pool(name="b_ps", bufs=1, space="PSUM"))
            cipool = _stk.enter_context(tc.tile_pool(name="c_idx", bufs=2))
            cgpool = _stk.enter_context(tc.tile_pool(name="c_g", bufs=3))
            cspool = _stk.enter_context(tc.tile_pool(name="c_s", bufs=2))
            cpspool = _stk.enter_context(tc.tile_pool(name="c_ps", bufs=1, space="PSUM"))

            # ---------------- phase Y: Y = x @ Wq_all ----------------
            with nc.named_scope("phaseY"):
                for g in range(TA):
                    yx_t = ypool.tile([P, P], FP16, tag="yx")
                    nc.sync.dma_start(out=yx_t[:],
                                      in_=xT16[:, g * P:(g + 1) * P])
                    y_ps = yps.tile([P, K * VEC], F32, tag="yps")
                    nc.tensor.matmul(out=y_ps[:], lhsT=yx_t[:], rhs=wq_sb[:],
                                     start=True, stop=True)
                    y_sb_t = ysb.tile([P, K * VEC], FP16, tag="ysb")
                    nc.vector.tensor_copy(out=y_sb_t[:], in_=y_ps[:])
                    eng = nc.scalar if g % 2 else nc.sync
                    eng.dma_start(
                        out=Yf[g * P * K:(g + 1) * P * K, :].rearrange(
                            "(p k) v -> p (k v)", p=P),
                        in_=y_sb_t[:])

            # ---------------- phase A: q + choice on own shard ----------------
            with nc.named_scope("phaseA"):
                if True:
                    for t in range(TO):
                        KT = kts[t]
                        idxa_t = ipool.tile([P, KT], I32)
                        nc.sync.dma_start(out=idxa_t[:],
                                          in_=idxa_own[t * P:(t + 1) * P, 0:KT])
                        qacc = tpool.tile([P, VEC], F32, tag="qacc")
                        for k in range(KT):
                            yg = gpool.tile([P, VEC], FP16, tag="yg")
                            nc.gpsimd.indirect_dma_start(
                                out=yg[:], out_offset=None, in_=Yf[:, :],
                                in_offset=bass.IndirectOffsetOnAxis(
                                    ap=idxa_t[:, k:k + 1], axis=0))
                            if k == 0:
                                nc.vector.tensor_copy(out=qacc[:], in_=yg[:])
                            else:
                                nc.vector.tensor_tensor(
                                    out=qacc[:], in0=qacc[:], in1=yg[:],
                                    op=mybir.AluOpType.add)
                        q_ps = pspool.tile([VEC, P], F32, tag="qT")
                        nc.tensor.matmul(out=q_ps[:], lhsT=qacc[:],
                                         rhs=ident32[:], start=True, stop=True)
                        qf = tpool.tile([VEC, P], F32, tag="qf")
                        nc.scalar.activation(
                            out=qf[:], in_=q_ps[:],
                            func=mybir.ActivationFunctionType.Relu,
                            bias=qb_sb[:, 0:1], scale=qg_sb[:, 0:1])
                        t_ps = ps2pool.tile([P, P], F32, tag="tch")
                        nc.tensor.matmul(out=t_ps[:], lhsT=qf[:], rhs=wcc_sb[:],
                                         start=True, stop=False)
                        nc.tensor.matmul(out=t_ps[:], lhsT=ones_row[:],
                                         rhs=bch_sb[:], start=False, stop=True)
                        scratch = tpool.tile([P, P], F32, tag="scr")
                        nc.scalar.activation(
                            out=scratch[:], in_=t_ps[:],
                            func=mybir.ActivationFunctionType.Relu,
                            accum_out=strip[:, t:t + 1])

            # ---------------- allgather choice ----------------
            with nc.named_scope("gather_choice"):
                nc.sync.dma_start(out=cc_in[:], in_=strip[:])
                nc.gpsimd.collective_compute(
                    "AllGather", mybir.AluOpType.bypass,
                    replica_groups=[list(range(NCORE))],
                    ins=[cc_in.opt()], outs=[cc_out.opt()])
                nc.sync.dma_start(
                    out=choice_all[:, 0:NCORE * TO].rearrange(
                        "p (r t) -> p r t", r=NCORE),
                    in_=cc_out[:, :, :].rearrange("r p t -> p r t"))

            # ---------------- phase B: build Tv table ----------------
            with nc.named_scope("phaseB"):
                if True:
                    for g in range(TA):
                        xt_t = bxpool.tile([P, P], FP16, tag="xt")
                        nc.sync.dma_start(out=xt_t[:],
                                          in_=xT16[:, g * P:(g + 1) * P])
                        c4_t = bxpool.tile([4, P], F32, tag="c4")
                        nc.sync.dma_start(out=c4_t[:],
                                          in_=coordsT4[:, g * P:(g + 1) * P])
                        v_ps = bpspool.tile([P, P], F32, tag="vps")
                        nc.tensor.matmul(out=v_ps[:], lhsT=xt_t[:], rhs=wv_sb[:],
                                         start=True, stop=False)
                        nc.tensor.matmul(out=v_ps[:], lhsT=ones_row[:],
                                         rhs=vbeta_sb[:], start=False, stop=True)
                        p_ps = bpspool.tile([P, VEC], F32, tag="pps")
                        nc.tensor.matmul(out=p_ps[:], lhsT=c4_t[:],
                                         rhs=wpos4_sb[:], start=True, stop=True)
                        tv_t = btvpool.tile([P, D], F32, tag="tv")
                        nc.scalar.activation(
                            out=tv_t[:, 0:P], in_=v_ps[:],
                            func=mybir.ActivationFunctionType.Relu)
                        pos_bc = bass.AP(p_ps.tensor, p_ps[:].offset,
                                         [p_ps[:].ap[0], (1, VEC), (0, P // VEC)])
                        nc.vector.tensor_tensor(
                            out=tv_t[:, 0:P], in0=tv_t[:, 0:P], in1=pos_bc,
                            op=mybir.AluOpType.add)
                        nc.vector.memset(tv_t[:, P:D], 0.0)
                        nc.scalar.dma_start(out=Tv[g * P:(g + 1) * P, :],
                                            in_=tv_t[:])

            # ---------------- phase B2: patch choice column into Tv ----------------
            with nc.named_scope("phaseB2"):
                GRP = 16
                for g0 in range(0, TA, GRP):
                    gn = min(GRP, TA - g0)
                    dst = bass.AP(Tv.tensor, g0 * P * D + P,
                                  [(D, P), (D * P, gn)])
                    nc.sync.dma_start(out=dst,
                                      in_=choice_all[:, g0:g0 + gn])

            # ---------------- phase C: scores, softmax, aggregate, out ----------------
            with nc.named_scope("phaseC"):
                if True:
                    for t in range(TO):
                        KT = kts[t]
                        idx_t = cipool.tile([P, KT], I32, tag="idx")
                        nc.sync.dma_start(out=idx_t[:],
                                          in_=idx_own[t * P:(t + 1) * P, 0:KT])
                        bias_t = cipool.tile([P, KT], F32, tag="bias")
                        nc.sync.dma_start(out=bias_t[:],
                                          in_=bias_own[t * P:(t + 1) * P, 0:KT])
                        g_all = cgpool.tile([P, KT * D], F32, tag="gall")
                        for k in range(KT):
                            nc.gpsimd.indirect_dma_start(
                                out=g_all[:, k * D:(k + 1) * D],
                                out_offset=None, in_=Tv[:, :],
                                in_offset=bass.IndirectOffsetOnAxis(
                                    ap=idx_t[:, k:k + 1], axis=0))
                        chg = g_all[:].rearrange("p (k d) -> p k d", k=KT)[:, :, P]
                        s_t = cspool.tile([P, KT], F32, tag="s")
                        nc.vector.scalar_tensor_tensor(
                            out=s_t[:], in0=chg, scalar=strip[:, t:t + 1],
                            in1=bias_t[:], op0=mybir.AluOpType.mult,
                            op1=mybir.AluOpType.add)
                        negmax = cspool.tile([P, 1], F32, tag="nm")
                        nc.vector.tensor_reduce(
                            out=negmax[:], in_=s_t[:], axis=mybir.AxisListType.X,
                            op=mybir.AluOpType.max, negate=True)
                        e_t = cspool.tile([P, KT], F32, tag="e")
                        esum = cspool.tile([P, 1], F32, tag="es")
                        nc.scalar.activation(
                            out=e_t[:], in_=s_t[:],
                            func=mybir.ActivationFunctionType.Exp,
                            bias=negmax[:, 0:1], scale=1.0,
                            accum_out=esum[:, 0:1])
                        rs = cspool.tile([P, 1], F32, tag="rs")
                        nc.vector.reciprocal(out=rs[:], in_=esum[:])
                        w_t = cspool.tile([P, KT], F32, tag="w")
                        nc.vector.tensor_scalar_mul(out=w_t[:], in0=e_t[:],
                                                    scalar1=rs[:, 0:1])
                        acc = cspool.tile([P, P], F32, tag="acc")
                        for k in range(KT):
                            vsl = g_all[:, k * D:k * D + P]
                            if k == 0:
                                nc.vector.tensor_scalar_mul(
                                    out=acc[:], in0=vsl, scalar1=w_t[:, 0:1])
                            else:
                                nc.vector.scalar_tensor_tensor(
                                    out=acc[:], in0=vsl, scalar=w_t[:, k:k + 1],
                                    in1=acc[:], op0=mybir.AluOpType.mult,
                                    op1=mybir.AluOpType.add)
                        tr2 = cpspool.tile([P, P], F32, tag="tr2")
                        nc.tensor.transpose(out=tr2[:], in_=acc[:],
                                            identity=ident32[:])
                        aggT = cspool.tile([P, P], F32, tag="aggT")
                        nc.vector.tensor_copy(out=aggT[:], in_=tr2[:])
                        o_ps = cpspool.tile([P, P], F32, tag="ops")
                        nc.tensor.matmul(out=o_ps[:], lhsT=wo_sb[:], rhs=aggT[:],
                                         start=True, stop=True)
                        oT = cspool.tile([P, P], F32, tag="oT")
                        nc.scalar.activation(
                            out=oT[:], in_=o_ps[:],
                            func=mybir.ActivationFunctionType.Relu,
                            bias=obeta_sb[:, 0:1], scale=1.0)
                        xo_t = cspool.tile([P, P], F32, tag="xo")
                        nc.sync.dma_start(out=xo_t[:],
                                          in_=xT_own[:, t * P:(t + 1) * P])
                        res_t = cspool.tile([P, P], F32, tag="res")
                        nc.vector.tensor_tensor(out=res_t[:], in0=oT[:],
                                                in1=xo_t[:],
                                                op=mybir.AluOpType.add)
                        nc.scalar.dma_start(out=outT[:, t * P:(t + 1) * P],
                                            in_=res_t[:])
            _stk.close()

    nc.finalize()
    return nc


def _prep(inputs):
    x = np.asarray(inputs["x"], np.float32)
    coords = np.asarray(inputs["coords"], np.float32)
    W_q = np.asarray(inputs["W_q"], np.float32)
    q_gamma = np.asarray(inputs["q_gamma"], np.float32)
    q_beta = np.asarray(inputs["q_beta"], np.float32)
    W_v = np.asarray(inputs["W_v"], np.float32)
    v_gamma = np.asarray(inputs["v_gamma"], np.float32)
    v_beta = np.asarray(inputs["v_beta"], np.float32)
    codebook = np.asarray(inputs["codebook"], np.float32)
    W_choice = np.asarray(inputs["W_choice"], np.float32)
    b_choice = np.asarray(inputs["b_choice"], np.float32)
    W_pos = np.asarray(inputs["W_pos"], np.float32)
    b_pos = np.asarray(inputs["b_pos"], np.float32)
    W_out = np.asarray(inputs["W_out"], np.float32)
    out_gamma = np.asarray(inputs["out_gamma"], np.float32)
    out_beta = np.asarray(inputs["out_beta"], np.float32)
    nbr_idx = np.asarray(inputs["nbr_idx"], np.int32)
    nbr_mask = np.asarray(inputs["nbr_mask"], np.int32)

    n = x.shape[0]
    assert n == N

    NTOT = NCORE * NSH                    # 100352 padded rows
    # ---- valid-degree sort (per core shard) → global relabeling ----
    mask_pad = np.zeros((K, NTOT), bool)
    mask_pad[:, :n] = nbr_mask > 0
    deg = mask_pad.sum(0)
    orders = []
    degs_sorted = np.empty((NCORE, NSH), np.int64)
    for r in range(NCORE):
        sl = slice(r * NSH, (r + 1) * NSH)
        o = np.argsort(-deg[sl], kind="stable")
        orders.append(o)
        degs_sorted[r] = deg[sl][o]
    kts = tuple(int(max(1, degs_sorted[:, t * P:(t + 1) * P].max()))
                for t in range(TO))
    perm_full = np.concatenate([r * NSH + orders[r] for r in range(NCORE)])
    inv = np.empty(NTOT, np.int64)
    inv[perm_full] = np.arange(NTOT)

    # ---- permuted global tables ----
    xp = np.zeros((NTOT, P), np.float32)
    xp[:n] = x
    xp2 = xp[perm_full]
    cp = np.zeros((NTOT, 3), np.float32)
    cp[:n] = coords
    cp2 = cp[perm_full]

    xT16 = np.ascontiguousarray(xp2[:PAD_N].T.astype(np.float16))
    coordsT4 = np.ones((4, PAD_N), np.float32)
    coordsT4[:3] = cp2[:PAD_N].T

    # ---- weight folds ----
    cb2 = float(np.dot(codebook, codebook))
    scb = np.sqrt(cb2).astype(np.float32)
    wcp = codebook[:, None] * W_choice
    wcc = scb * wcp.reshape(VEC, P // VEC, P).sum(1)
    bch = (scb * b_choice)[None, :]
    wv = (W_v * v_gamma[None, :]).astype(np.float16)
    wpos4 = np.concatenate([W_pos, b_pos[None, :]], axis=0)
    wq_flat = np.ascontiguousarray(
        W_q.transpose(1, 0, 2).reshape(P, K * VEC)).astype(np.float16)
    wo = W_out * out_gamma[None, :]

    # ---- per-slot idx/bias in NEW row ids, compacted valid-first ----
    idx_new = np.full((K, NTOT), Z, np.int32)
    idx_new[:, :n] = np.where(nbr_mask > 0, inv[nbr_idx], Z).astype(np.int32)
    bias_pad = np.full((K, NTOT), np.float32(NEG), np.float32)
    bias_pad[:, :n] = np.where(nbr_mask > 0, 0.0, NEG).astype(np.float32)
    korder = np.argsort(~mask_pad, axis=0, kind="stable")   # valid ks first
    idx_new = np.take_along_axis(idx_new, korder, axis=0)
    bias_pad = np.take_along_axis(bias_pad, korder, axis=0)
    # phase-A flat Y indices: neighbor_row*27 + original k (Z*27 for padding)
    idxa = np.where(idx_new != Z, idx_new.astype(np.int64) * K + korder,
                    Z * K).astype(np.int32)
    # permute slot-grid columns to sorted point order
    idx_new = idx_new[:, perm_full]
    bias_pad = bias_pad[:, perm_full]
    idxa = idxa[:, perm_full]

    shared = dict(xT16=xT16, coordsT4=coordsT4,
                  w_q=wq_flat,
                  qg=q_gamma[:, None], qb=q_beta[:, None],
                  wcc=wcc, bch=bch, wv=wv,
                  vbeta=v_beta[None, :],
                  wpos4=wpos4, wo=wo, obeta=out_beta[:, None])
    in_maps = []
    for r in range(NCORE):
        sl = slice(r * NSH, (r + 1) * NSH)
        m = dict(shared)
        m["xT_own"] = np.ascontiguousarray(xp2[sl].T)
        m["idx_own"] = np.ascontiguousarray(idx_new[:, sl].T)
        m["idxa_own"] = np.ascontiguousarray(idxa[:, sl].T)
        m["bias_own"] = np.ascontiguousarray(bias_pad[:, sl].T)
        in_maps.append(m)
    return in_maps, kts, orders


def prepare(inputs):
    in_maps, kts, orders = _prep(inputs)
    if _CACHE.get("kts") != kts:
        _CACHE["nc"] = _build_nc(kts)
        _CACHE["kts"] = kts
    return _CACHE["nc"], in_maps, orders


def assemble(results, orders):
    out = np.empty((NCORE * NSH, P), np.float32)
    for r in range(NCORE):
        out[r * NSH + orders[r]] = results[r]["outT"].T
    return np.ascontiguousarray(out[:N])


def kernel(**inputs):
    nc, in_maps, orders = prepare(inputs)
    res = run_bass_kernel_spmd(nc, in_maps, list(range(NCORE)))
    return assemble(res.results, orders)


if __name__ == "__main__":
    rng = np.random.default_rng(0)
    ins = dict(
        x=rng.standard_normal((N, P)).astype(np.float32),
        coords=(rng.random((N, 3)) * 100).astype(np.float32),
        W_q=rng.standard_normal((K, P, VEC)).astype(np.float32) * (P * K) ** -0.5,
        q_gamma=np.ones(VEC, np.float32), q_beta=np.zeros(VEC, np.float32),
        W_v=rng.standard_normal((P, P)).astype(np.float32) * P ** -0.5,
        v_gamma=np.ones(P, np.float32), v_beta=np.zeros(P, np.float32),
        codebook=rng.standard_normal(P).astype(np.float32) * 0.1,
        W_choice=rng.standard_normal((P, P)).astype(np.float32) * P ** -0.5,
        b_choice=np.zeros(P, np.float32),
        W_pos=rng.standard_normal((3, VEC)).astype(np.float32) * 3 ** -0.5,
        b_pos=np.zeros(VEC, np.float32),
        W_out=rng.standard_normal((P, P)).astype(np.float32) * P ** -0.5,
        out_gamma=np.ones(P, np.float32), out_beta=np.zeros(P, np.float32),
        nbr_idx=rng.integers(0, N, (K, N)).astype(np.int32),
        nbr_mask=rng.integers(0, 2, (K, N)).astype(np.int32),
    )
    out = kernel(**ins)
    print("kernel output", out.shape, out.dtype)


# revision 23
# speedup vs baseline: 1.1833x; 1.0168x over previous
"""Trainium2 Bass kernel for nn_DiscreteQKTRBlock (sparse 3x3x3 neighborhood
attention with a discrete codebook).

Strategy (data-parallel over points, 8 cores):

The reference's discrete-codebook STE path collapses algebraically:
    dq[i]   = codebook * choice[i]           (forward value of the STE)
    s[k,i]  = dq[i] . dq[nbr[k,i]] = ||codebook||^2 * choice[i] * choice[nbr[k,i]]
so the [N,128] per-offset dot products reduce to scalar products of a single
[N] vector `choice`.  Phases:

  A) each core computes q (sparse 27-offset conv via indirect row gathers of
     x), then choice' = sqrt(cb2)*choice for its 12544-point shard
  B) AllGather choice' (50KB/core); every core builds the full value table
     Tv[i] = [v_f(i) (128) | choice'(i)]  (v_f = relu(x@Wv*g+b)+pos)
  C) gather Tv rows for the 27 neighbors of each own point, masked softmax
     over offsets, weighted accumulation, output projection + residual.

Masking is folded in host-side: masked/padded neighbors get index Z=100000
which points at an all-zero table row, plus a -1e9 additive score bias.

All weight-affine folds (gamma/beta, codebook scaling into W_choice, bias
rows appended to coords) are host-side weight-space transforms only.
"""
import sys
sys.path.insert(0, "/opt/trn_rl_repo")
import numpy as np
import ml_dtypes

from concourse import bass, bacc, mybir
import concourse.tile as tile
from concourse.bass_utils import run_bass_kernel_spmd
from concourse.masks import make_identity

F32 = mybir.dt.float32
BF16 = mybir.dt.bfloat16
FP16 = mybir.dt.float16
I32 = mybir.dt.int32

N = 100000
P = 128
VEC = 16
K = 27
NEG = -1e9
NCORE = 8
NSH = 12544                # points per core (98 tiles of 128)
TO = NSH // P              # 98 own tiles
PAD_N = 100096             # 782 * 128  (full table rows incl. zero row)
TA = PAD_N // P            # 782 build tiles
Z = N                      # zero-row index for masked/padded neighbors
D = 129                    # Tv row: 128 v_f + 1 choice

_CACHE = {}


def _build_nc(kts):
    nc = bacc.Bacc(num_devices=NCORE, dynamic_dma_scratch_size=131072)

    # ---------------- inputs ----------------
    xT16 = nc.declare_dram_parameter("xT16", [P, PAD_N], FP16, isOutput=False)
    coordsT4 = nc.declare_dram_parameter("coordsT4", [4, PAD_N], F32, isOutput=False)
    xT_own = nc.declare_dram_parameter("xT_own", [P, NSH], F32, isOutput=False)
    idx_own = nc.declare_dram_parameter("idx_own", [NSH, K], I32, isOutput=False)
    idxa_own = nc.declare_dram_parameter("idxa_own", [NSH, K], I32, isOutput=False)
    bias_own = nc.declare_dram_parameter("bias_own", [NSH, K], F32, isOutput=False)
    w_q = nc.declare_dram_parameter("w_q", [P, K * VEC], FP16, isOutput=False)
    qg_in = nc.declare_dram_parameter("qg", [VEC, 1], F32, isOutput=False)
    qb_in = nc.declare_dram_parameter("qb", [VEC, 1], F32, isOutput=False)
    wcc_in = nc.declare_dram_parameter("wcc", [VEC, P], F32, isOutput=False)
    bch_in = nc.declare_dram_parameter("bch", [1, P], F32, isOutput=False)
    wv_in = nc.declare_dram_parameter("wv", [P, P], FP16, isOutput=False)
    vbeta_in = nc.declare_dram_parameter("vbeta", [1, P], F32, isOutput=False)
    wpos4_in = nc.declare_dram_parameter("wpos4", [4, VEC], F32, isOutput=False)
    wo_in = nc.declare_dram_parameter("wo", [P, P], F32, isOutput=False)
    obeta_in = nc.declare_dram_parameter("obeta", [P, 1], F32, isOutput=False)

    outT = nc.declare_dram_parameter("outT", [P, NSH], F32, isOutput=True)

    with tile.TileContext(nc) as tc:
        with tc.tile_pool(name="const", bufs=1) as cpool, \
             tc.tile_pool(name="work", bufs=1) as wpool, \
             tc.tile_pool(name="dram", bufs=1, space="DRAM") as dpool:

            # resident constants
            wq_sb = cpool.tile([P, K * VEC], FP16)
            nc.sync.dma_start(out=wq_sb[:], in_=w_q[:, :])
            qg_sb = cpool.tile([VEC, 1], F32)
            nc.sync.dma_start(out=qg_sb[:], in_=qg_in[:, :])
            qb_sb = cpool.tile([VEC, 1], F32)
            nc.sync.dma_start(out=qb_sb[:], in_=qb_in[:, :])
            wcc_sb = cpool.tile([VEC, P], F32)
            nc.sync.dma_start(out=wcc_sb[:], in_=wcc_in[:, :])
            bch_sb = cpool.tile([1, P], F32)
            nc.sync.dma_start(out=bch_sb[:], in_=bch_in[:, :])
            wv_sb = cpool.tile([P, P], FP16)
            nc.sync.dma_start(out=wv_sb[:], in_=wv_in[:, :])
            vbeta_sb = cpool.tile([1, P], F32)
            nc.sync.dma_start(out=vbeta_sb[:], in_=vbeta_in[:, :])
            wpos4_sb = cpool.tile([4, VEC], F32)
            nc.sync.dma_start(out=wpos4_sb[:], in_=wpos4_in[:, :])
            wo_sb = cpool.tile([P, P], F32)
            nc.sync.dma_start(out=wo_sb[:], in_=wo_in[:, :])
            obeta_sb = cpool.tile([P, 1], F32)
            nc.sync.dma_start(out=obeta_sb[:], in_=obeta_in[:, :])

            ident32 = cpool.tile([P, P], F32)
            make_identity(nc, ident32[:])


            ones_row = cpool.tile([1, P], F32)
            nc.vector.memset(ones_row[:], 1.0)

            strip = cpool.tile([P, TO], F32)        # own choice' per tile col
            choice_all = cpool.tile([P, TA + 2], F32)  # wrapped full choice'

            Tv = dpool.tile([PAD_N, D], F32)
            Yf = dpool.tile([PAD_N * K, VEC], FP16)
            cc_in = dpool.tile([P, TO], F32)
            cc_out = dpool.tile([NCORE, P, TO], F32, addr_space="Shared")

            # ---------------- all per-phase pools (opened up-front so phases overlap) ----
            from contextlib import ExitStack
            _stk = ExitStack()
            ipool = _stk.enter_context(tc.tile_pool(name="a_idx", bufs=2))
            gpool = _stk.enter_context(tc.tile_pool(name="a_xg", bufs=32))
            tpool = _stk.enter_context(tc.tile_pool(name="a_xgT", bufs=6))
            pspool = _stk.enter_context(tc.tile_pool(name="a_ps", bufs=2, space="PSUM"))
            ypool = _stk.enter_context(tc.tile_pool(name="y_x", bufs=6))
            ysb = _stk.enter_context(tc.tile_pool(name="y_sb", bufs=10))
            yps = _stk.enter_context(tc.tile_pool(name="y_ps", bufs=1, space="PSUM"))
            bxpool = _stk.enter_context(tc.tile_pool(name="b_x", bufs=4))
            btvpool = _stk.enter_context(tc.tile_pool(name="b_tv", bufs=10))
            bpspool = _stk.enter_context(tc.tile_pool(name="b_ps", bufs=2, space="PSUM"))
            bps2pool = _stk.enter_context(tc.tile_pool(name="b_ps2", bufs=1, space="PSUM"))
            cipool = _stk.enter_context(tc.tile_pool(name="c_idx", bufs=2))
            cgpool = _stk.enter_context(tc.tile_pool(name="c_g", bufs=3))
            cspool = _stk.enter_context(tc.tile_pool(name="c_s", bufs=2))
            cpspool = _stk.enter_context(tc.tile_pool(name="c_ps", bufs=2, space="PSUM"))

            # ---------------- phase Y: Y = x @ Wq_all ----------------
            with nc.named_scope("phaseY"):
                for g in range(TA):
                    yx_t = ypool.tile([P, P], FP16, tag="yx")
                    nc.sync.dma_start(out=yx_t[:],
                                      in_=xT16[:, g * P:(g + 1) * P])
                    y_ps = yps.tile([P, K * VEC], F32, tag="yps")
                    nc.tensor.matmul(out=y_ps[:], lhsT=yx_t[:], rhs=wq_sb[:],
                                     start=True, stop=True)
                    y_sb_t = ysb.tile([P, K * VEC], FP16, tag="ysb")
                    nc.vector.tensor_copy(out=y_sb_t[:], in_=y_ps[:])
                    eng = nc.scalar if g % 2 else nc.sync
                    eng.dma_start(
                        out=Yf[g * P * K:(g + 1) * P * K, :].rearrange(
                            "(p k) v -> p (k v)", p=P),
                        in_=y_sb_t[:])

            # ---------------- phase A: q + choice on own shard ----------------
            with nc.named_scope("phaseA"):
                if True:
                    for t in range(TO):
                        KT = kts[t]
                        idxa_t = ipool.tile([P, KT], I32)
                        nc.sync.dma_start(out=idxa_t[:],
                                          in_=idxa_own[t * P:(t + 1) * P, 0:KT])
                        qacc = tpool.tile([P, VEC], F32, tag="qacc")
                        for k in range(KT):
                            yg = gpool.tile([P, VEC], FP16, tag="yg")
                            nc.gpsimd.indirect_dma_start(
                                out=yg[:], out_offset=None, in_=Yf[:, :],
                                in_offset=bass.IndirectOffsetOnAxis(
                                    ap=idxa_t[:, k:k + 1], axis=0))
                            if k == 0:
                                nc.vector.tensor_copy(out=qacc[:], in_=yg[:])
                            else:
                                nc.vector.tensor_tensor(
                                    out=qacc[:], in0=qacc[:], in1=yg[:],
                                    op=mybir.AluOpType.add)
                        q_ps = pspool.tile([VEC, P], F32, tag="qT", padded_shape=[P, P])
                        nc.tensor.matmul(out=q_ps[:], lhsT=qacc[:],
                                         rhs=ident32[:], start=True, stop=True)
                        qf = tpool.tile([VEC, P], F32, tag="qf")
                        nc.scalar.activation(
                            out=qf[:], in_=q_ps[:],
                            func=mybir.ActivationFunctionType.Relu,
                            bias=qb_sb[:, 0:1], scale=qg_sb[:, 0:1])
                        t_ps = pspool.tile([P, P], F32, tag="qT")
                        nc.tensor.matmul(out=t_ps[:], lhsT=qf[:], rhs=wcc_sb[:],
                                         start=True, stop=False)
                        nc.tensor.matmul(out=t_ps[:], lhsT=ones_row[:],
                                         rhs=bch_sb[:], start=False, stop=True)
                        scratch = tpool.tile([P, P], F32, tag="scr")
                        nc.scalar.activation(
                            out=scratch[:], in_=t_ps[:],
                            func=mybir.ActivationFunctionType.Relu,
                            accum_out=strip[:, t:t + 1])

            # ---------------- allgather choice ----------------
            with nc.named_scope("gather_choice"):
                nc.sync.dma_start(out=cc_in[:], in_=strip[:])
                nc.gpsimd.collective_compute(
                    "AllGather", mybir.AluOpType.bypass,
                    replica_groups=[list(range(NCORE))],
                    ins=[cc_in.opt()], outs=[cc_out.opt()])
                nc.sync.dma_start(
                    out=choice_all[:, 0:NCORE * TO].rearrange(
                        "p (r t) -> p r t", r=NCORE),
                    in_=cc_out[:, :, :].rearrange("r p t -> p r t"))

            # ---------------- phase B: build Tv table ----------------
            with nc.named_scope("phaseB"):
                if True:
                    for g in range(TA):
                        xt_t = bxpool.tile([P, P], FP16, tag="xt")
                        nc.sync.dma_start(out=xt_t[:],
                                          in_=xT16[:, g * P:(g + 1) * P])
                        c4_t = bxpool.tile([4, P], F32, tag="c4")
                        nc.sync.dma_start(out=c4_t[:],
                                          in_=coordsT4[:, g * P:(g + 1) * P])
                        v_ps = bpspool.tile([P, P], F32, tag="vps")
                        nc.tensor.matmul(out=v_ps[:], lhsT=xt_t[:], rhs=wv_sb[:],
                                         start=True, stop=False)
                        nc.tensor.matmul(out=v_ps[:], lhsT=ones_row[:],
                                         rhs=vbeta_sb[:], start=False, stop=True)
                        p_ps = bps2pool.tile([P, VEC], F32, tag="pps")
                        nc.tensor.matmul(out=p_ps[:], lhsT=c4_t[:],
                                         rhs=wpos4_sb[:], start=True, stop=True)
                        tv_t = btvpool.tile([P, D], F32, tag="tv")
                        nc.scalar.activation(
                            out=tv_t[:, 0:P], in_=v_ps[:],
                            func=mybir.ActivationFunctionType.Relu)
                        pos_bc = bass.AP(p_ps.tensor, p_ps[:].offset,
                                         [p_ps[:].ap[0], (1, VEC), (0, P // VEC)])
                        nc.vector.tensor_tensor(
                            out=tv_t[:, 0:P], in0=tv_t[:, 0:P], in1=pos_bc,
                            op=mybir.AluOpType.add)
                        nc.vector.memset(tv_t[:, P:D], 0.0)
                        nc.scalar.dma_start(out=Tv[g * P:(g + 1) * P, :],
                                            in_=tv_t[:])

            # ---------------- phase B2: patch choice column into Tv ----------------
            with nc.named_scope("phaseB2"):
                GRP = 16
                for g0 in range(0, TA, GRP):
                    gn = min(GRP, TA - g0)
                    dst = bass.AP(Tv.tensor, g0 * P * D + P,
                                  [(D, P), (D * P, gn)])
                    nc.sync.dma_start(out=dst,
                                      in_=choice_all[:, g0:g0 + gn])

            # ---------------- phase C: scores, softmax, aggregate, out ----------------
            with nc.named_scope("phaseC"):
                if True:
                    for t in range(TO):
                        KT = kts[t]
                        idx_t = cipool.tile([P, KT], I32, tag="idx")
                        nc.sync.dma_start(out=idx_t[:],
                                          in_=idx_own[t * P:(t + 1) * P, 0:KT])
                        bias_t = cipool.tile([P, KT], F32, tag="bias")
                        nc.sync.dma_start(out=bias_t[:],
                                          in_=bias_own[t * P:(t + 1) * P, 0:KT])
                        g_all = cgpool.tile([P, KT * D], F32, tag="gall")
                        for k in range(KT):
                            nc.gpsimd.indirect_dma_start(
                                out=g_all[:, k * D:(k + 1) * D],
                                out_offset=None, in_=Tv[:, :],
                                in_offset=bass.IndirectOffsetOnAxis(
                                    ap=idx_t[:, k:k + 1], axis=0))
                        chg = g_all[:].rearrange("p (k d) -> p k d", k=KT)[:, :, P]
                        s_t = cspool.tile([P, KT], F32, tag="s")
                        nc.vector.scalar_tensor_tensor(
                            out=s_t[:], in0=chg, scalar=strip[:, t:t + 1],
                            in1=bias_t[:], op0=mybir.AluOpType.mult,
                            op1=mybir.AluOpType.add)
                        negmax = cspool.tile([P, 1], F32, tag="nm")
                        nc.vector.tensor_reduce(
                            out=negmax[:], in_=s_t[:], axis=mybir.AxisListType.X,
                            op=mybir.AluOpType.max, negate=True)
                        e_t = cspool.tile([P, KT], F32, tag="e")
                        esum = cspool.tile([P, 1], F32, tag="es")
                        nc.scalar.activation(
                            out=e_t[:], in_=s_t[:],
                            func=mybir.ActivationFunctionType.Exp,
                            bias=negmax[:, 0:1], scale=1.0,
                            accum_out=esum[:, 0:1])
                        rs = cspool.tile([P, 1], F32, tag="rs")
                        nc.vector.reciprocal(out=rs[:], in_=esum[:])
                        w_t = cspool.tile([P, KT], F32, tag="w")
                        nc.vector.tensor_scalar_mul(out=w_t[:], in0=e_t[:],
                                                    scalar1=rs[:, 0:1])
                        acc = cspool.tile([P, P], F32, tag="acc")
                        for k in range(KT):
                            vsl = g_all[:, k * D:k * D + P]
                            if k == 0:
                                nc.vector.tensor_scalar_mul(
                                    out=acc[:], in0=vsl, scalar1=w_t[:, 0:1])
                            else:
                                nc.vector.scalar_tensor_tensor(
                                    out=acc[:], in0=vsl, scalar=w_t[:, k:k + 1],
                                    in1=acc[:], op0=mybir.AluOpType.mult,
                                    op1=mybir.AluOpType.add)
                        tr2 = cpspool.tile([P, P], F32, tag="cps")
                        nc.tensor.transpose(out=tr2[:], in_=acc[:],
                                            identity=ident32[:])
                        aggT = cspool.tile([P, P], F32, tag="aggT")
                        nc.vector.tensor_copy(out=aggT[:], in_=tr2[:])
                        o_ps = cpspool.tile([P, P], F32, tag="cps")
                        nc.tensor.matmul(out=o_ps[:], lhsT=wo_sb[:], rhs=aggT[:],
                                         start=True, stop=True)
                        oT = cspool.tile([P, P], F32, tag="oT")
                        nc.scalar.activation(
                            out=oT[:], in_=o_ps[:],
                            func=mybir.ActivationFunctionType.Relu,
                            bias=obeta_sb[:, 0:1], scale=1.0)
                        xo_t = cspool.tile([P, P], F32, tag="xo")
                        nc.sync.dma_start(out=xo_t[:],
                                          in_=xT_own[:, t * P:(t + 1) * P])
                        res_t = cspool.tile([P, P], F32, tag="res")
                        nc.vector.tensor_tensor(out=res_t[:], in0=oT[:],
                                                in1=xo_t[:],
                                                op=mybir.AluOpType.add)
                        nc.scalar.dma_start(out=outT[:, t * P:(t + 1) * P],
                                            in_=res_t[:])
            _stk.close()

    nc.finalize()
    return nc


def _prep(inputs):
    x = np.asarray(inputs["x"], np.float32)
    coords = np.asarray(inputs["coords"], np.float32)
    W_q = np.asarray(inputs["W_q"], np.float32)
    q_gamma = np.asarray(inputs["q_gamma"], np.float32)
    q_beta = np.asarray(inputs["q_beta"], np.float32)
    W_v = np.asarray(inputs["W_v"], np.float32)
    v_gamma = np.asarray(inputs["v_gamma"], np.float32)
    v_beta = np.asarray(inputs["v_beta"], np.float32)
    codebook = np.asarray(inputs["codebook"], np.float32)
    W_choice = np.asarray(inputs["W_choice"], np.float32)
    b_choice = np.asarray(inputs["b_choice"], np.float32)
    W_pos = np.asarray(inputs["W_pos"], np.float32)
    b_pos = np.asarray(inputs["b_pos"], np.float32)
    W_out = np.asarray(inputs["W_out"], np.float32)
    out_gamma = np.asarray(inputs["out_gamma"], np.float32)
    out_beta = np.asarray(inputs["out_beta"], np.float32)
    nbr_idx = np.asarray(inputs["nbr_idx"], np.int32)
    nbr_mask = np.asarray(inputs["nbr_mask"], np.int32)

    n = x.shape[0]
    assert n == N

    NTOT = NCORE * NSH                    # 100352 padded rows
    # ---- valid-degree sort (per core shard) → global relabeling ----
    mask_pad = np.zeros((K, NTOT), bool)
    mask_pad[:, :n] = nbr_mask > 0
    deg = mask_pad.sum(0)
    orders = []
    degs_sorted = np.empty((NCORE, NSH), np.int64)
    for r in range(NCORE):
        sl = slice(r * NSH, (r + 1) * NSH)
        o = np.argsort(-deg[sl], kind="stable")
        orders.append(o)
        degs_sorted[r] = deg[sl][o]
    kts = tuple(int(max(1, degs_sorted[:, t * P:(t + 1) * P].max()))
                for t in range(TO))
    perm_full = np.concatenate([r * NSH + orders[r] for r in range(NCORE)])
    inv = np.empty(NTOT, np.int64)
    inv[perm_full] = np.arange(NTOT)

    # ---- permuted global tables ----
    xp = np.zeros((NTOT, P), np.float32)
    xp[:n] = x
    xp2 = xp[perm_full]
    cp = np.zeros((NTOT, 3), np.float32)
    cp[:n] = coords
    cp2 = cp[perm_full]

    xT16 = np.ascontiguousarray(xp2[:PAD_N].T.astype(np.float16))
    coordsT4 = np.ones((4, PAD_N), np.float32)
    coordsT4[:3] = cp2[:PAD_N].T

    # ---- weight folds ----
    cb2 = float(np.dot(codebook, codebook))
    scb = np.sqrt(cb2).astype(np.float32)
    wcp = codebook[:, None] * W_choice
    wcc = scb * wcp.reshape(VEC, P // VEC, P).sum(1)
    bch = (scb * b_choice)[None, :]
    wv = (W_v * v_gamma[None, :]).astype(np.float16)
    wpos4 = np.concatenate([W_pos, b_pos[None, :]], axis=0)
    wq_flat = np.ascontiguousarray(
        W_q.transpose(1, 0, 2).reshape(P, K * VEC)).astype(np.float16)
    wo = W_out * out_gamma[None, :]

    # ---- per-slot idx/bias in NEW row ids, compacted valid-first ----
    idx_new = np.full((K, NTOT), Z, np.int32)
    idx_new[:, :n] = np.where(nbr_mask > 0, inv[nbr_idx], Z).astype(np.int32)
    bias_pad = np.full((K, NTOT), np.float32(NEG), np.float32)
    bias_pad[:, :n] = np.where(nbr_mask > 0, 0.0, NEG).astype(np.float32)
    korder = np.argsort(~mask_pad, axis=0, kind="stable")   # valid ks first
    idx_new = np.take_along_axis(idx_new, korder, axis=0)
    bias_pad = np.take_along_axis(bias_pad, korder, axis=0)
    # phase-A flat Y indices: neighbor_row*27 + original k (Z*27 for padding)
    idxa = np.where(idx_new != Z, idx_new.astype(np.int64) * K + korder,
                    Z * K).astype(np.int32)
    # permute slot-grid columns to sorted point order
    idx_new = idx_new[:, perm_full]
    bias_pad = bias_pad[:, perm_full]
    idxa = idxa[:, perm_full]

    shared = dict(xT16=xT16, coordsT4=coordsT4,
                  w_q=wq_flat,
                  qg=q_gamma[:, None], qb=q_beta[:, None],
                  wcc=wcc, bch=bch, wv=wv,
                  vbeta=v_beta[None, :],
                  wpos4=wpos4, wo=wo, obeta=out_beta[:, None])
    in_maps = []
    for r in range(NCORE):
        sl = slice(r * NSH, (r + 1) * NSH)
        m = dict(shared)
        m["xT_own"] = np.ascontiguousarray(xp2[sl].T)
        m["idx_own"] = np.ascontiguousarray(idx_new[:, sl].T)
        m["idxa_own"] = np.ascontiguousarray(idxa[:, sl].T)
        m["bias_own"] = np.ascontiguousarray(bias_pad[:, sl].T)
        in_maps.append(m)
    return in_maps, kts, orders


def prepare(inputs):
    in_maps, kts, orders = _prep(inputs)
    if _CACHE.get("kts") != kts:
        _CACHE["nc"] = _build_nc(kts)
        _CACHE["kts"] = kts
    return _CACHE["nc"], in_maps, orders


def assemble(results, orders):
    out = np.empty((NCORE * NSH, P), np.float32)
    for r in range(NCORE):
        out[r * NSH + orders[r]] = results[r]["outT"].T
    return np.ascontiguousarray(out[:N])


def kernel(**inputs):
    nc, in_maps, orders = prepare(inputs)
    res = run_bass_kernel_spmd(nc, in_maps, list(range(NCORE)))
    return assemble(res.results, orders)


if __name__ == "__main__":
    rng = np.random.default_rng(0)
    ins = dict(
        x=rng.standard_normal((N, P)).astype(np.float32),
        coords=(rng.random((N, 3)) * 100).astype(np.float32),
        W_q=rng.standard_normal((K, P, VEC)).astype(np.float32) * (P * K) ** -0.5,
        q_gamma=np.ones(VEC, np.float32), q_beta=np.zeros(VEC, np.float32),
        W_v=rng.standard_normal((P, P)).astype(np.float32) * P ** -0.5,
        v_gamma=np.ones(P, np.float32), v_beta=np.zeros(P, np.float32),
        codebook=rng.standard_normal(P).astype(np.float32) * 0.1,
        W_choice=rng.standard_normal((P, P)).astype(np.float32) * P ** -0.5,
        b_choice=np.zeros(P, np.float32),
        W_pos=rng.standard_normal((3, VEC)).astype(np.float32) * 3 ** -0.5,
        b_pos=np.zeros(VEC, np.float32),
        W_out=rng.standard_normal((P, P)).astype(np.float32) * P ** -0.5,
        out_gamma=np.ones(P, np.float32), out_beta=np.zeros(P, np.float32),
        nbr_idx=rng.integers(0, N, (K, N)).astype(np.int32),
        nbr_mask=rng.integers(0, 2, (K, N)).astype(np.int32),
    )
    out = kernel(**ins)
    print("kernel output", out.shape, out.dtype)
